# revision 1
# baseline (speedup 1.0000x reference)
"""AttnBlock (GroupNorm -> QKV 1x1 conv -> spatial attention with softmax over
query-H axis -> output projection + residual) for B=8, C=128, H=W=48 on 8
Trainium2 NeuronCores, data-parallel over batch (1 batch per core).

Math per batch (N = H*W = 2304 spatial positions, C = 128 channels):
  xn = GroupNorm(x; 32 groups of 4 channels)
  q/k/v = W @ xn + b              (per-position 1x1 conv = C x C matmul)
  S[q', kp] = q[:,q'] . k[:,kp] / sqrt(C)
  attn = softmax over the query-H axis: for fixed (w, kp), normalize over h
  ctx[c, (h,w)] = sum_kp attn[(h,w), kp] * v[c, kp]
  out = x + Wo @ ctx + bo

Device mapping:
  - Channels on the 128 SBUF partitions; spatial positions on the free axis.
  - S computed transposed (S^T [kp, q']) per 128-key chunk so the softmax
    reduction (over h) runs along the free axis (VectorE grouped reduce).
  - Queries stored w-major (q' = w*48 + h) so each softmax group of 48 h
    values is contiguous; the reorder is free (permuted APs on the projection
    evacuations).
  - Projections in float32r; q/k/v rounded to bf16 once at evacuation, so the
    attention matmuls run at full bf16 rate with fast weight loads.
  - All 18 normalized-E (bf16) chunk tiles stay resident in SBUF; the S^T
    staging PSUM pool gets two slots so TensorE/ScalarE ping-pong freely.
  - The GroupNorm affine is folded into the projection weights (no separate
    normalize pass over x).
  - ctx accumulates in 4 PSUM banks for query columns 0:2048 (interleaved,
    lagged four chunks behind the softmax chain); the 256-column tail gets a
    short dense pass at the end.
  - Normalize-muls split between GpSimd (13 chunks) and VectorE (5 chunks);
    the residual add runs per output block on VectorE.
"""

import sys

sys.path.insert(0, "/opt/trn_rl_repo")

import numpy as np

import concourse.bass as bass
import concourse.mybir as mybir
import concourse.tile as tile
from concourse import bacc, bass_utils

B, C, H, W = 8, 128, 48, 48
N = H * W  # 2304
GROUPS = 32
GSIZE = C // GROUPS
EPS = 1e-5
NCORES = 8

F32 = mybir.dt.float32
F32R = mybir.dt.float32r
BF16 = mybir.dt.bfloat16
AF = mybir.ActivationFunctionType
OP = mybir.AluOpType

NCHUNK = N // 128  # 18 key chunks
QG = 768  # S^T staging / exp granularity
NQG = N // QG  # 3
CTX_LIVE = [0, 512, 1024, 1536]  # 4 psum-resident ctx banks (512 wide each)
TAIL_OFF, TAIL_SZ = 2048, 256  # final ctx region, computed in a tail pass
DVE_MUL_CHUNKS = {0, 4, 8, 12, 16}  # normalize-mul on VectorE; rest on GpSimd


def _build_program():
    nc = bacc.Bacc("TRN2", target_bir_lowering=False, debug=False)

    def din(name, shape, dt=F32):
        return nc.dram_tensor(name, shape, dt, kind="ExternalInput")

    x_d = din("x", [C, N], F32R)
    gnw_d = din("gn_w", [C, 1])
    gnb_d = din("gn_b", [C, 1])
    wqT_d = din("wqT", [C, C], F32R)
    wkT_d = din("wkT", [C, C], F32R)
    wvT_d = din("wvT", [C, C], F32R)
    woT_d = din("woT", [C, C], F32R)
    bq_d = din("bq", [C, 1])
    bk_d = din("bk", [C, 1])
    bv_d = din("bv", [C, 1])
    bo_d = din("bo", [C, 1])
    gmat_d = din("gmat", [C, GROUPS], F32R)
    gexp_d = din("gexp", [GROUPS, C], F32R)
    ident_d = din("ident", [C, C], BF16)
    out_d = nc.dram_tensor("out", [C, N], F32, kind="ExternalOutput")

    with tile.TileContext(nc) as tc:
        with (
            tc.tile_pool(name="const", bufs=1) as const,
            tc.tile_pool(name="data", bufs=1) as data,
            tc.tile_pool(name="small", bufs=1) as small,
            tc.tile_pool(name="soft", bufs=6) as soft,
            tc.tile_pool(name="epool", bufs=NCHUNK) as epool,
        ):
            # ---- input loads (x first: GroupNorm depends only on it) ----
            tx = data.tile([C, N], F32R)
            nc.sync.dma_start(tx[:], x_d[:])
            txf = tx[:].bitcast(F32)

            wqT = const.tile([C, C], F32R)
            wkT = const.tile([C, C], F32R)
            wvT = const.tile([C, C], F32R)
            woT = const.tile([C, C], F32R)
            gmat = const.tile([C, GROUPS], F32R)
            gexp = const.tile([GROUPS, C], F32R)
            ident = const.tile([C, C], BF16)
            gnw = const.tile([C, 1], F32)
            gnb = const.tile([C, 1], F32)
            bq = const.tile([C, 1], F32)
            bk = const.tile([C, 1], F32)
            bv = const.tile([C, 1], F32)
            bo = const.tile([C, 1], F32)
            for t, d in [
                (gmat, gmat_d), (gexp, gexp_d), (gnw, gnw_d), (gnb, gnb_d),
                (wqT, wqT_d), (wkT, wkT_d), (wvT, wvT_d), (woT, woT_d),
                (ident, ident_d),
                (bq, bq_d), (bk, bk_d), (bv, bv_d), (bo, bo_d),
            ]:
                nc.sync.dma_start(t[:], d[:])

            # ---- GroupNorm statistics ----
            sq_scratch = data.tile([C, N], F32)
            stats_f = small.tile([C, 2], F32)
            nc.vector.tensor_reduce(
                stats_f[:, 0:1], txf, axis=mybir.AxisListType.X, op=OP.add
            )
            nc.scalar.activation(
                sq_scratch[:], txf, AF.Square, accum_out=stats_f[:, 1:2]
            )
            stats = small.tile([C, 2], F32R)
            nc.vector.tensor_copy(stats[:], stats_f[:])

            with tc.tile_pool(name="gnps", bufs=1, space="PSUM") as gnps:
                psg = gnps.tile([GROUPS, 2], F32)
                nc.tensor.matmul(psg[:], gmat[:], stats[:], start=True, stop=True)

                inv_n = 1.0 / (GSIZE * N)
                t32 = small.tile([GROUPS, 4], F32)
                nc.vector.tensor_scalar_mul(t32[:, 0:1], psg[:, 0:1], inv_n)
                nc.vector.tensor_scalar_mul(t32[:, 1:2], psg[:, 1:2], inv_n)
                nc.vector.tensor_mul(t32[:, 2:3], t32[:, 0:1], t32[:, 0:1])
                nc.vector.tensor_sub(t32[:, 3:4], t32[:, 1:2], t32[:, 2:3])
                eps_t = small.tile([GROUPS, 1], F32)
                nc.vector.memset(eps_t[:], EPS)
                nc.scalar.activation(t32[:, 2:3], t32[:, 3:4], AF.Ln, bias=eps_t[:])
                rstd_f = small.tile([GROUPS, 1], F32)
                nc.scalar.activation(rstd_f[:], t32[:, 2:3], AF.Exp, scale=-0.5)
                mstat = small.tile([GROUPS, 2], F32R)
                nc.vector.tensor_copy(mstat[:, 0:1], t32[:, 0:1])
                nc.vector.tensor_copy(mstat[:, 1:2], rstd_f[:])

                pse = gnps.tile([C, 2], F32)
                nc.tensor.matmul(pse[:], gexp[:], mstat[:], start=True, stop=True)

                A_sb = small.tile([C, 1], F32)
                B_sb = small.tile([C, 1], F32)
                nc.vector.tensor_mul(A_sb[:], pse[:, 1:2], gnw[:])
                nc.vector.tensor_mul(B_sb[:], pse[:, 0:1], A_sb[:])
                nc.vector.tensor_sub(B_sb[:], gnb[:], B_sb[:])

            # ---- fold the GroupNorm affine into the projection weights:
            # ---- q = Wq(A*x + B) + bq = (Wq diag(A)) x + (Wq B + bq)
            wq2 = small.tile([C, C], F32R)
            wk2 = small.tile([C, C], F32R)
            wv2 = small.tile([C, C], F32R)
            bq2 = small.tile([C, 1], F32)
            bk2 = small.tile([C, 1], F32)
            bv2 = small.tile([C, 1], F32)
            with tc.tile_pool(name="foldps", bufs=1, space="PSUM") as foldps:
                psb = foldps.tile([C, 4], F32)
                for i, (wT, w2, bias, b2) in enumerate((
                    (wqT, wq2, bq, bq2),
                    (wkT, wk2, bk, bk2),
                    (wvT, wv2, bv, bv2),
                )):
                    nc.vector.tensor_scalar_mul(w2[:], wT[:], A_sb[:])
                    nc.tensor.matmul(
                        psb[:, i : i + 1], wT[:].bitcast(F32), B_sb[:],
                        start=True, stop=True,
                    )
                    nc.vector.tensor_add(b2[:], psb[:, i : i + 1], bias[:])

            # ---- Q/K/V projections (q written w-major); q,k first so the
            # ---- attention loop can start while v/vT still evacuates ----
            q = data.tile([C, N], BF16)
            k = data.tile([C, N], BF16)
            v = data.tile([C, N], BF16)
            q_wh = q[:].rearrange("p (w h) -> p h w", h=H)
            with tc.tile_pool(name="projps", bufs=2, space="PSUM") as projps:
                for g in range(NQG):
                    for wT, bias, dst, permute in (
                        (wq2, bq2, q, True),
                        (wk2, bk2, k, False),
                        (wv2, bv2, v, False),
                    ):
                        pp = projps.tile([C, QG], F32, tag="pp")
                        o = g * QG
                        nc.tensor.matmul(
                            pp[:, 0:512], wT[:], tx[:, o : o + 512],
                            start=True, stop=True,
                        )
                        nc.tensor.matmul(
                            pp[:, 512:QG], wT[:], tx[:, o + 512 : o + QG],
                            start=True, stop=True,
                        )
                        if permute:
                            outv = q_wh[:, 16 * g : 16 * (g + 1), :]
                            inv = pp[:].rearrange("p (h w) -> p h w", w=W)
                        else:
                            outv = dst[:, o : o + QG]
                            inv = pp[:, :]
                        nc.scalar.activation(outv, inv, AF.Identity, bias=bias[:])

                vT = data.tile([C, NCHUNK * C], BF16)
                for grp in range(0, NCHUNK, 4):
                    cnt = min(4, NCHUNK - grp)
                    pvt = projps.tile([C, 512], BF16, tag="pvt")
                    for j in range(cnt):
                        ch = grp + j
                        nc.tensor.transpose(
                            pvt[:, 128 * j : 128 * (j + 1)],
                            v[:, 128 * ch : 128 * (ch + 1)],
                            ident[:],
                        )
                    nc.vector.tensor_copy(
                        vT[:, 128 * grp : 128 * (grp + cnt)], pvt[:, : 128 * cnt]
                    )

            # ---- main attention loop ----
            e_tiles = [None] * NCHUNK
            ctx_all = data.tile([C, N], F32R)
            with (
                tc.tile_pool(name="ctxps", bufs=1, space="PSUM") as ctxps,
                tc.tile_pool(name="sps", bufs=2, space="PSUM") as sps,
            ):
                ctx_ps = [
                    ctxps.tile([C, 512], F32, tag=f"ctx{i}", name=f"ctx_ps{i}")
                    for i in range(len(CTX_LIVE))
                ]

                def emit_av(ch, part):
                    ec = e_tiles[ch]
                    for i in ([0, 1], [2], [3])[part]:
                        o = CTX_LIVE[i]
                        nc.tensor.matmul(
                            ctx_ps[i][:, :],
                            vT[:, 128 * ch : 128 * (ch + 1)],
                            ec[:, o : o + 512],
                            start=(ch == 0),
                            stop=(ch == NCHUNK - 1),
                        )

                for it in range(NCHUNK + 4):
                    ch = it if it < NCHUNK else None
                    av = it - 4
                    if ch is not None:
                        ec = epool.tile([C, N], BF16, tag="E", name=f"E_{ch}")
                        e_tiles[ch] = ec
                        klhs = k[:, 128 * ch : 128 * (ch + 1)]
                        for g in range(NQG):
                            ps = sps.tile([C, QG], F32, tag="spsum")
                            o = g * QG
                            nc.tensor.matmul(
                                ps[:, 0:512], klhs, q[:, o : o + 512],
                                start=True, stop=True,
                            )
                            nc.tensor.matmul(
                                ps[:, 512:QG], klhs, q[:, o + 512 : o + QG],
                                start=True, stop=True,
                            )
                            nc.scalar.activation(ec[:, o : o + QG], ps[:, :], AF.Exp)
                            if av >= 0:
                                emit_av(av, g)
                    else:
                        for g in range(NQG):
                            emit_av(av, g)

                    if ch is None:
                        continue
                    dsum = soft.tile([C, W], F32, tag="D")
                    nc.vector.tensor_reduce(
                        dsum[:],
                        ec[:].rearrange("p (w h) -> p w h", h=H),
                        axis=mybir.AxisListType.X,
                        op=OP.add,
                    )
                    rden = soft.tile([C, W], F32, tag="R")
                    nc.vector.reciprocal_approx_fast(rden[:], dsum[:])
                    ev = ec[:].rearrange("p (w h) -> p w h", h=H)
                    if ch in DVE_MUL_CHUNKS:
                        nc.vector.tensor_tensor(
                            out=ev, in0=ev,
                            in1=rden[:, :, None].to_broadcast([C, W, H]),
                            op=OP.mult,
                        )
                    else:
                        rden_b = soft.tile([C, W], BF16, tag="Rb")
                        nc.vector.tensor_copy(rden_b[:], rden[:])
                        nc.gpsimd.tensor_tensor(
                            out=ev, in0=ev,
                            in1=rden_b[:, :, None].to_broadcast([C, W, H]),
                            op=OP.mult,
                        )

                for i, o in enumerate(CTX_LIVE):
                    nc.scalar.copy(ctx_all[:, o : o + 512], ctx_ps[i][:, :])

            # ---- ctx tail (columns 2048:2304) + output projection + residual ----
            out_nat = data.tile([C, N], F32)
            out_wh = out_nat[:].rearrange("p (h w) -> p w h", w=W)
            with tc.tile_pool(name="ops", bufs=2, space="PSUM") as ops:
                def outproj(g):
                    po = ops.tile([C, QG], F32, tag="po", name=f"po_{g}")
                    o = g * QG
                    nc.tensor.matmul(
                        po[:, 0:512], woT[:], ctx_all[:, o : o + 512],
                        start=True, stop=True,
                    )
                    nc.tensor.matmul(
                        po[:, 512:QG], woT[:], ctx_all[:, o + 512 : o + QG],
                        start=True, stop=True,
                    )
                    ov = out_wh[:, 16 * g : 16 * (g + 1), :]
                    nc.scalar.activation(
                        ov,
                        po[:].rearrange("p (w h) -> p w h", h=H),
                        AF.Identity,
                        bias=bo[:],
                    )
                    txv = txf.rearrange("p (h w) -> p w h", w=W)
                    nc.vector.tensor_tensor(
                        out=ov, in0=ov,
                        in1=txv[:, 16 * g : 16 * (g + 1), :],
                        op=OP.add,
                    )

                outproj(0)
                outproj(1)
                tail = ops.tile([C, TAIL_SZ], F32, tag="tail")
                for ch in range(NCHUNK):
                    nc.tensor.matmul(
                        tail[:, :],
                        vT[:, 128 * ch : 128 * (ch + 1)],
                        e_tiles[ch][:, TAIL_OFF : TAIL_OFF + TAIL_SZ],
                        start=(ch == 0),
                        stop=(ch == NCHUNK - 1),
                    )
                nc.scalar.copy(ctx_all[:, TAIL_OFF : TAIL_OFF + TAIL_SZ], tail[:, :])
                outproj(2)
            nc.sync.dma_start(out_d[:], out_nat[:])

    nc.compile()
    return nc


_PROGRAM_CACHE = None


def kernel(**inputs: np.ndarray) -> np.ndarray:
    global _PROGRAM_CACHE
    if _PROGRAM_CACHE is None:
        _PROGRAM_CACHE = _build_program()
    nc = _PROGRAM_CACHE

    import ml_dtypes

    f32 = lambda a: np.ascontiguousarray(np.asarray(a), dtype=np.float32)
    x = f32(inputs["x"])
    scale = 1.0 / np.sqrt(np.float32(C))

    gmat = np.zeros((C, GROUPS), np.float32)
    gmat[np.arange(C), np.arange(C) // GSIZE] = 1.0

    shared = {
        "gn_w": f32(inputs["gn_w"]).reshape(C, 1),
        "gn_b": f32(inputs["gn_b"]).reshape(C, 1),
        "wqT": np.ascontiguousarray(f32(inputs["wq"]).T * scale),
        "wkT": np.ascontiguousarray(f32(inputs["wk"]).T),
        "wvT": np.ascontiguousarray(f32(inputs["wv"]).T),
        "woT": np.ascontiguousarray(f32(inputs["wo"]).T),
        "bq": f32(inputs["bq"]).reshape(C, 1) * scale,
        "bk": f32(inputs["bk"]).reshape(C, 1),
        "bv": f32(inputs["bv"]).reshape(C, 1),
        "bo": f32(inputs["bo"]).reshape(C, 1),
        "gmat": gmat,
        "gexp": np.ascontiguousarray(gmat.T),
        "ident": np.eye(C).astype(ml_dtypes.bfloat16),
    }
    in_maps = [
        {**shared, "x": np.ascontiguousarray(x[b].reshape(C, N))} for b in range(B)
    ]

    res = bass_utils.run_bass_kernel_spmd(nc, in_maps, core_ids=list(range(NCORES)))
    out = np.stack([res.results[b]["out"].reshape(C, H, W) for b in range(B)])
    return out.astype(np.float32)



# revision 5
# speedup vs baseline: 1.1181x; 1.1181x over previous
"""AttnBlock (GroupNorm -> QKV 1x1 conv -> spatial attention with softmax over
query-H axis -> output projection + residual) for B=8, C=128, H=W=48 on 8
Trainium2 NeuronCores, data-parallel over batch (1 batch per core).

Math per batch (N = H*W = 2304 spatial positions, C = 128 channels):
  xn = GroupNorm(x; 32 groups of 4 channels)
  q/k/v = W @ xn + b              (per-position 1x1 conv = C x C matmul)
  S[q', kp] = q[:,q'] . k[:,kp] / sqrt(C)
  attn = softmax over the query-H axis: for fixed (w, kp), normalize over h
  ctx[c, (h,w)] = sum_kp attn[(h,w), kp] * v[c, kp]
  out = x + Wo @ ctx + bo

Device mapping (v2 — natural layout + engine rebalance):
  - Channels on the 128 SBUF partitions; spatial positions on the free axis
    in NATURAL (h, w) order everywhere (q' = h*48 + w).  The softmax group
    (fixed w, varying h) is then stride-48 in the free axis: the VectorE
    grouped reduce handles that at the same 1x cost, while the normalize
    multiply gets a contiguous inner dim (w) and runs at 2x DVE rate, and
    every projection evacuation becomes a contiguous write (no permuted APs,
    which cost ~2.2x on ScalarE).
  - S computed transposed (S^T [kp, q']) per 128-key chunk; exp on ScalarE
    at 768 granularity (PSUM bank limit); E chunks stay resident in SBUF
    (bf16) so the attention matmuls run at full bf16 rate.
  - Softmax denominators: grouped reduce split between VectorE tensor_reduce
    and GpSimd avg-pool (InstPool is in the gpsimd standard library); the
    1/48 from avg-pool is folded into the bf16 cast of the reciprocal.
  - Normalize-muls all on VectorE (2x mode: bf16, inner step-1 broadcast).
  - GroupNorm statistics via one VectorE bn_stats pass (mean/var per
    channel), group-combined with tiny matmuls; affine folded into the
    projection weights.
  - ctx accumulates in 4 PSUM banks for columns 0:2048 (lagged four chunks
    behind the softmax chain); the 256-column tail gets a short dense pass
    at the end, overlapped with the output projection.
  - Output projection evacuation fused with bias + residual add in a single
    VectorE scalar_tensor_tensor per 768 columns; output DMA split per
    768-column group to overlap the epilogue.
"""

import sys

sys.path.insert(0, "/opt/trn_rl_repo")

import numpy as np

import concourse.bass as bass
import concourse.mybir as mybir
import concourse.tile as tile
from concourse import bacc, bass_utils

B, C, H, W = 8, 128, 48, 48
N = H * W  # 2304
GROUPS = 32
GSIZE = C // GROUPS
EPS = 1e-5
NCORES = 8

F32 = mybir.dt.float32
F32R = mybir.dt.float32r
BF16 = mybir.dt.bfloat16
AF = mybir.ActivationFunctionType
OP = mybir.AluOpType

NCHUNK = N // 128  # 18 key chunks
QG = 768  # S^T staging / exp granularity
NQG = N // QG  # 3
CTX_LIVE = [0, 512, 1024, 1536]  # 4 psum-resident ctx banks (512 wide each)
TAIL_OFF, TAIL_SZ = 2048, 256  # final ctx region, computed in a tail pass
MUL_VE_CHUNKS = {1, 3, 5, 8, 10, 12, 14, 16}  # normalize-mul on VectorE; rest GpSimd


def _build_program():
    nc = bacc.Bacc("TRN2", target_bir_lowering=False, debug=False)

    def din(name, shape, dt=F32):
        return nc.dram_tensor(name, shape, dt, kind="ExternalInput")

    x_d = din("x", [C, N], F32R)
    gnw_d = din("gn_w", [C, 1])
    gnb_d = din("gn_b", [C, 1])
    wqT_d = din("wqT", [C, C], F32R)
    wkT_d = din("wkT", [C, C], F32R)
    wvT_d = din("wvT", [C, C], F32R)
    woT_d = din("woT", [C, C], F32R)
    bq_d = din("bq", [C, 1])
    bk_d = din("bk", [C, 1])
    bv_d = din("bv", [C, 1])
    bo_d = din("bo", [C, 1])
    gmat_d = din("gmat", [C, GROUPS], F32R)
    gexp_d = din("gexp", [GROUPS, C], F32R)
    ident_d = din("ident", [C, C], BF16)
    out_d = nc.dram_tensor("out", [C, N], F32, kind="ExternalOutput")

    with tile.TileContext(nc) as tc:
        with (
            tc.tile_pool(name="const", bufs=1) as const,
            tc.tile_pool(name="data", bufs=1) as data,
            tc.tile_pool(name="small", bufs=1) as small,
            tc.tile_pool(name="soft", bufs=6) as soft,
            tc.tile_pool(name="epool", bufs=NCHUNK) as epool,
        ):
            # ---- input loads (x first: GroupNorm depends only on it) ----
            tx = data.tile([C, N], F32R)
            nc.sync.dma_start(tx[:], x_d[:])
            txf = tx[:].bitcast(F32)

            wqT = const.tile([C, C], F32R)
            wkT = const.tile([C, C], F32R)
            wvT = const.tile([C, C], F32R)
            woT = const.tile([C, C], F32R)
            gmat = const.tile([C, GROUPS], F32R)
            gexp = const.tile([GROUPS, C], F32R)
            ident = const.tile([C, C], BF16)
            gnw = const.tile([C, 1], F32)
            gnb = const.tile([C, 1], F32)
            bq = const.tile([C, 1], F32)
            bk = const.tile([C, 1], F32)
            bv = const.tile([C, 1], F32)
            bo = const.tile([C, 1], F32)
            for t, d in [
                (gmat, gmat_d), (gexp, gexp_d), (gnw, gnw_d), (gnb, gnb_d),
                (wqT, wqT_d), (wkT, wkT_d), (wvT, wvT_d), (woT, woT_d),
                (ident, ident_d),
                (bq, bq_d), (bk, bk_d), (bv, bv_d), (bo, bo_d),
            ]:
                nc.sync.dma_start(t[:], d[:])

            # ---- GroupNorm statistics: one bn_stats pass for mean/var ----
            bstats = small.tile([C, 6, 6], F32)
            for i in range(6):
                nc.vector.bn_stats(bstats[:, i, :], txf[:, 384 * i : 384 * (i + 1)])
            mv = small.tile([C, 2], F32)  # per-channel (mean, var)
            nc.vector.bn_aggr(mv[:], bstats[:].rearrange("p g f -> p (g f)"))

            # per-channel (mean, E[x^2]); group-combine via gmat matmul
            mex = small.tile([C, 2], F32)
            nc.vector.tensor_mul(mex[:, 1:2], mv[:, 0:1], mv[:, 0:1])
            nc.vector.tensor_add(mex[:, 1:2], mex[:, 1:2], mv[:, 1:2])
            nc.vector.tensor_copy(mex[:, 0:1], mv[:, 0:1])
            mex_r = small.tile([C, 2], F32R)
            nc.vector.tensor_copy(mex_r[:], mex[:])

            with tc.tile_pool(name="gnps", bufs=1, space="PSUM") as gnps:
                psg = gnps.tile([GROUPS, 2], F32)
                nc.tensor.matmul(psg[:], gmat[:], mex_r[:], start=True, stop=True)

                inv_g = 1.0 / GSIZE
                t32 = small.tile([GROUPS, 4], F32)
                nc.vector.tensor_scalar_mul(t32[:, 0:1], psg[:, 0:1], inv_g)
                nc.vector.tensor_scalar_mul(t32[:, 1:2], psg[:, 1:2], inv_g)
                nc.vector.tensor_mul(t32[:, 2:3], t32[:, 0:1], t32[:, 0:1])
                nc.vector.tensor_sub(t32[:, 3:4], t32[:, 1:2], t32[:, 2:3])
                eps_t = small.tile([GROUPS, 1], F32)
                nc.vector.memset(eps_t[:], EPS)
                nc.scalar.activation(t32[:, 2:3], t32[:, 3:4], AF.Ln, bias=eps_t[:])
                rstd_f = small.tile([GROUPS, 1], F32)
                nc.scalar.activation(rstd_f[:], t32[:, 2:3], AF.Exp, scale=-0.5)
                mstat = small.tile([GROUPS, 2], F32R)
                nc.vector.tensor_copy(mstat[:, 0:1], t32[:, 0:1])
                nc.vector.tensor_copy(mstat[:, 1:2], rstd_f[:])

                pse = gnps.tile([C, 2], F32)
                nc.tensor.matmul(pse[:], gexp[:], mstat[:], start=True, stop=True)

                A_sb = small.tile([C, 1], F32)
                B_sb = small.tile([C, 1], F32)
                nc.vector.tensor_mul(A_sb[:], pse[:, 1:2], gnw[:])
                nc.vector.tensor_mul(B_sb[:], pse[:, 0:1], A_sb[:])
                nc.vector.tensor_sub(B_sb[:], gnb[:], B_sb[:])

            # ---- fold the GroupNorm affine into the projection weights:
            # ---- q = Wq(A*x + B) + bq = (Wq diag(A)) x + (Wq B + bq)
            wq2 = small.tile([C, C], F32R)
            wk2 = small.tile([C, C], F32R)
            wv2 = small.tile([C, C], F32R)
            bq2 = small.tile([C, 1], F32)
            bk2 = small.tile([C, 1], F32)
            bv2 = small.tile([C, 1], F32)
            with tc.tile_pool(name="foldps", bufs=1, space="PSUM") as foldps:
                psb = foldps.tile([C, 4], F32)
                for i, (wT, w2, bias, b2) in enumerate((
                    (wqT, wq2, bq, bq2),
                    (wkT, wk2, bk, bk2),
                    (wvT, wv2, bv, bv2),
                )):
                    nc.vector.tensor_scalar_mul(w2[:], wT[:], A_sb[:])
                    nc.tensor.matmul(
                        psb[:, i : i + 1], wT[:].bitcast(F32), B_sb[:],
                        start=True, stop=True,
                    )
                    nc.vector.tensor_add(b2[:], psb[:, i : i + 1], bias[:])

            # ---- Q/K/V projections (all natural (h,w) order); q,k first so
            # ---- the attention loop can start while v/vT still evacuates ----
            q = data.tile([C, N], BF16)
            k = data.tile([C, N], BF16)
            v = data.tile([C, N], BF16)
            with tc.tile_pool(name="projps", bufs=2, space="PSUM") as projps:
                for g in range(NQG):
                    for wT, bias, dst, eng in (
                        (wq2, bq2, q, "scalar"),
                        (wk2, bk2, k, "scalar"),
                        (wv2, bv2, v, "vector"),
                    ):
                        pp = projps.tile([C, QG], F32, tag="pp")
                        o = g * QG
                        nc.tensor.matmul(
                            pp[:, 0:512], wT[:], tx[:, o : o + 512],
                            start=True, stop=True,
                        )
                        nc.tensor.matmul(
                            pp[:, 512:QG], wT[:], tx[:, o + 512 : o + QG],
                            start=True, stop=True,
                        )
                        outv = dst[:, o : o + QG]
                        if eng == "scalar":
                            nc.scalar.activation(outv, pp[:], AF.Identity, bias=bias[:])
                        else:
                            nc.vector.tensor_scalar(
                                outv, pp[:], bias[:], None, op0=OP.add
                            )

                vT = data.tile([C, NCHUNK * C], BF16)
                for grp in range(0, NCHUNK, 4):
                    cnt = min(4, NCHUNK - grp)
                    pvt = projps.tile([C, 512], BF16, tag="pvt")
                    for j in range(cnt):
                        ch = grp + j
                        nc.tensor.transpose(
                            pvt[:, 128 * j : 128 * (j + 1)],
                            v[:, 128 * ch : 128 * (ch + 1)],
                            ident[:],
                        )
                    nc.vector.tensor_copy(
                        vT[:, 128 * grp : 128 * (grp + cnt)], pvt[:, : 128 * cnt]
                    )

            # ---- main attention loop ----
            e_tiles = [None] * NCHUNK
            ctx_all = data.tile([C, N], F32R)
            with (
                tc.tile_pool(name="ctxps", bufs=1, space="PSUM") as ctxps,
                tc.tile_pool(name="sps", bufs=2, space="PSUM") as sps,
            ):
                ctx_ps = [
                    ctxps.tile([C, 512], F32, tag=f"ctx{i}", name=f"ctx_ps{i}")
                    for i in range(len(CTX_LIVE))
                ]

                def emit_av(ch, part):
                    ec = e_tiles[ch]
                    for i in ([0, 1], [2], [3])[part]:
                        o = CTX_LIVE[i]
                        nc.tensor.matmul(
                            ctx_ps[i][:, :],
                            vT[:, 128 * ch : 128 * (ch + 1)],
                            ec[:, o : o + 512],
                            start=(ch == 0),
                            stop=(ch == NCHUNK - 1),
                        )

                for it in range(NCHUNK + 4):
                    ch = it if it < NCHUNK else None
                    av = it - 4
                    if ch is not None:
                        ec = epool.tile([C, N], BF16, tag="E", name=f"E_{ch}")
                        e_tiles[ch] = ec
                        klhs = k[:, 128 * ch : 128 * (ch + 1)]
                        for g in range(NQG):
                            ps = sps.tile([C, QG], F32, tag="spsum")
                            o = g * QG
                            nc.tensor.matmul(
                                ps[:, 0:512], klhs, q[:, o : o + 512],
                                start=True, stop=True,
                            )
                            nc.tensor.matmul(
                                ps[:, 512:QG], klhs, q[:, o + 512 : o + QG],
                                start=True, stop=True,
                            )
                            nc.scalar.activation(ec[:, o : o + QG], ps[:, :], AF.Exp)
                            if av >= 0:
                                emit_av(av, g)
                    else:
                        for g in range(NQG):
                            emit_av(av, g)

                    if ch is None:
                        continue
                    # grouped softmax reduce over h (stride-48 groups, w inner):
                    # stage 1 folds h 48->24 with a 2x-mode bf16 add, stage 2
                    # is the 1x grouped tensor_reduce on the half-size tile.
                    ev3 = ec[:].rearrange("p (h w) -> p h w", w=W)
                    ehalf = soft.tile([C, H // 2, W], BF16, tag="EH")
                    nc.vector.tensor_tensor(
                        out=ehalf[:], in0=ev3[:, 0 : H // 2, :],
                        in1=ev3[:, H // 2 : H, :], op=OP.add,
                    )
                    dsum = soft.tile([C, W], F32, tag="D")
                    nc.vector.tensor_reduce(
                        dsum[:],
                        ehalf[:].rearrange("p h w -> p w h"),
                        axis=mybir.AxisListType.X,
                        op=OP.add,
                    )
                    rden = soft.tile([C, W], F32, tag="R")
                    nc.vector.reciprocal_approx_fast(rden[:], dsum[:])
                    rden_b = soft.tile([C, W], BF16, tag="Rb")
                    nc.vector.tensor_copy(rden_b[:], rden[:])
                    # normalize: [p, h, w] *= rden_b[p, w] (bcast over h) — 2x DVE
                    rb3 = rden_b[:, None, :].to_broadcast([C, H, W])
                    if ch in MUL_VE_CHUNKS:
                        nc.vector.tensor_tensor(
                            out=ev3, in0=ev3, in1=rb3, op=OP.mult
                        )
                    else:
                        nc.gpsimd.tensor_tensor(
                            out=ev3, in0=ev3, in1=rb3, op=OP.mult
                        )

                for i, o in enumerate(CTX_LIVE):
                    nc.scalar.copy(ctx_all[:, o : o + 512], ctx_ps[i][:, :])

            # ---- ctx tail (columns 2048:2304) + output projection + residual ----
            out_nat = data.tile([C, N], F32)
            with tc.tile_pool(name="ops", bufs=2, space="PSUM") as ops:
                tail = ops.tile([C, TAIL_SZ], F32, tag="tail")
                for ch in range(NCHUNK):
                    nc.tensor.matmul(
                        tail[:, :],
                        vT[:, 128 * ch : 128 * (ch + 1)],
                        e_tiles[ch][:, TAIL_OFF : TAIL_OFF + TAIL_SZ],
                        start=(ch == 0),
                        stop=(ch == NCHUNK - 1),
                    )
                nc.scalar.copy(ctx_all[:, TAIL_OFF : TAIL_OFF + TAIL_SZ], tail[:, :])

                for g in range(NQG):
                    po = ops.tile([C, QG], F32, tag="po", name=f"po_{g}")
                    o = g * QG
                    nc.tensor.matmul(
                        po[:, 0:512], woT[:], ctx_all[:, o : o + 512],
                        start=True, stop=True,
                    )
                    nc.tensor.matmul(
                        po[:, 512:QG], woT[:], ctx_all[:, o + 512 : o + QG],
                        start=True, stop=True,
                    )
                    # fused bias + residual: out = (po + bo) + x
                    nc.vector.scalar_tensor_tensor(
                        out_nat[:, o : o + QG], po[:], bo[:],
                        txf[:, o : o + QG], op0=OP.add, op1=OP.add,
                    )
                    nc.sync.dma_start(
                        out_d[:, o : o + QG], out_nat[:, o : o + QG]
                    )

    nc.compile()
    return nc


_PROGRAM_CACHE = None


def kernel(**inputs: np.ndarray) -> np.ndarray:
    global _PROGRAM_CACHE
    if _PROGRAM_CACHE is None:
        _PROGRAM_CACHE = _build_program()
    nc = _PROGRAM_CACHE

    import ml_dtypes

    f32 = lambda a: np.ascontiguousarray(np.asarray(a), dtype=np.float32)
    x = f32(inputs["x"])
    scale = 1.0 / np.sqrt(np.float32(C))

    gmat = np.zeros((C, GROUPS), np.float32)
    gmat[np.arange(C), np.arange(C) // GSIZE] = 1.0

    shared = {
        "gn_w": f32(inputs["gn_w"]).reshape(C, 1),
        "gn_b": f32(inputs["gn_b"]).reshape(C, 1),
        "wqT": np.ascontiguousarray(f32(inputs["wq"]).T * scale),
        "wkT": np.ascontiguousarray(f32(inputs["wk"]).T),
        "wvT": np.ascontiguousarray(f32(inputs["wv"]).T),
        "woT": np.ascontiguousarray(f32(inputs["wo"]).T),
        "bq": f32(inputs["bq"]).reshape(C, 1) * scale,
        "bk": f32(inputs["bk"]).reshape(C, 1),
        "bv": f32(inputs["bv"]).reshape(C, 1),
        "bo": f32(inputs["bo"]).reshape(C, 1),
        "gmat": gmat,
        "gexp": np.ascontiguousarray(gmat.T),
        "ident": np.eye(C).astype(ml_dtypes.bfloat16),
    }
    in_maps = [
        {**shared, "x": np.ascontiguousarray(x[b].reshape(C, N))} for b in range(B)
    ]

    res = bass_utils.run_bass_kernel_spmd(nc, in_maps, core_ids=list(range(NCORES)))
    out = np.stack([res.results[b]["out"].reshape(C, H, W) for b in range(B)])
    return out.astype(np.float32)


# revision 7
# speedup vs baseline: 1.1564x; 1.0343x over previous
"""AttnBlock (GroupNorm -> QKV 1x1 conv -> spatial attention with softmax over
query-H axis -> output projection + residual) for B=8, C=128, H=W=48 on 8
Trainium2 NeuronCores, data-parallel over batch (1 batch per core).

Math per batch (N = H*W = 2304 spatial positions, C = 128 channels):
  xn = GroupNorm(x; 32 groups of 4 channels)
  q/k/v = W @ xn + b              (per-position 1x1 conv = C x C matmul)
  S[q', kp] = q[:,q'] . k[:,kp] / sqrt(C)
  attn = softmax over the query-H axis: for fixed (w, kp), normalize over h
  ctx[c, (h,w)] = sum_kp attn[(h,w), kp] * v[c, kp]
  out = x + Wo @ ctx + bo

Device mapping (v3 — host-side w-major relabeling + paired softmax):
  - The HOST permutes x to w-major (q' = w*48 + h) before upload and
    un-permutes the output after download, so on device every softmax group
    (fixed w, varying h) is 48 contiguous elements and every device-side
    access stays dense — no permuted APs anywhere in the kernel.
  - Channels on the 128 SBUF partitions; spatial positions on the free axis.
  - S computed transposed (S^T [kp, q']) per 128-key chunk; exp on ScalarE at
    768 granularity (PSUM bank limit); E chunks live in SBUF (bf16), two
    chunks per tile so softmax post-processing runs once per PAIR of chunks
    (halves the per-op DVE/semaphore overhead, which is ~0.3us per op).
  - Denominators: one VectorE grouped tensor_reduce per pair (contiguous
    inner axis, 1x is the DVE cap for reduce) + fast reciprocal.
  - Reciprocals duplicated into adjacent pairs (bf16) on ScalarE so the
    normalize-mul's broadcast operand has an innermost step-1 pair and the
    DVE runs it in 2x mode; normalize-muls split VectorE / GpSimd.
  - ctx accumulates in 4 PSUM banks for columns 0:2048 (lagged four chunks
    behind the softmax chain); the 256-column tail gets a short dense pass
    at the end, overlapped with the output projection.
  - GroupNorm statistics via bn_stats segments pipelined with the x DMA;
    affine folded into the projection weights.  Activation tables (Exp/Ln)
    are warmed with dummy ops at kernel start so their loads overlap DMA.
  - Output projection evacuation fused with bias + residual add in a single
    VectorE scalar_tensor_tensor per 768 columns; output DMA split per
    768-column group.
"""

import sys

sys.path.insert(0, "/opt/trn_rl_repo")

import numpy as np

import concourse.bass as bass
import concourse.mybir as mybir
import concourse.tile as tile
from concourse import bacc, bass_utils

B, C, H, W = 8, 128, 48, 48
N = H * W  # 2304
GROUPS = 32
GSIZE = C // GROUPS
EPS = 1e-5
NCORES = 8

F32 = mybir.dt.float32
F32R = mybir.dt.float32r
BF16 = mybir.dt.bfloat16
AF = mybir.ActivationFunctionType
OP = mybir.AluOpType

NCHUNK = N // 128  # 18 key chunks
NPAIR = NCHUNK // 2  # 9 softmax pairs
QG = 768  # S^T staging / exp granularity
NQG = N // QG  # 3
CTX_LIVE = [0, 512, 1024, 1536]  # 4 psum-resident ctx banks (512 wide each)
TAIL_OFF, TAIL_SZ = 2048, 256  # final ctx region, computed in a tail pass
MUL_VE_PAIRS = {1, 4, 7}  # normalize-mul on VectorE for these pairs; rest GpSimd


def _build_program():
    nc = bacc.Bacc("TRN2", target_bir_lowering=False, debug=False)

    def din(name, shape, dt=F32):
        return nc.dram_tensor(name, shape, dt, kind="ExternalInput")

    x_d = din("x", [C, N], F32R)
    gnw_d = din("gn_w", [C, 1])
    gnb_d = din("gn_b", [C, 1])
    wqT_d = din("wqT", [C, C], F32R)
    wkT_d = din("wkT", [C, C], F32R)
    wvT_d = din("wvT", [C, C], F32R)
    woT_d = din("woT", [C, C], F32R)
    bq_d = din("bq", [C, 1])
    bk_d = din("bk", [C, 1])
    bv_d = din("bv", [C, 1])
    bo_d = din("bo", [C, 1])
    gmat_d = din("gmat", [C, GROUPS], F32R)
    gexp_d = din("gexp", [GROUPS, C], F32R)
    ident_d = din("ident", [C, C], BF16)
    out_d = nc.dram_tensor("out", [C, N], F32, kind="ExternalOutput")

    NSEG = 6  # x DMA / bn_stats segments
    SEG = N // NSEG  # 384

    with tile.TileContext(nc) as tc:
        with (
            tc.tile_pool(name="const", bufs=1) as const,
            tc.tile_pool(name="data", bufs=1) as data,
            tc.tile_pool(name="small", bufs=1) as small,
            tc.tile_pool(name="soft", bufs=4) as soft,
            tc.tile_pool(name="epool", bufs=NPAIR) as epool,
        ):
            # ---- warm the Exp/Ln activation tables while DMAs run ----
            warm = small.tile([C, 2], F32)
            nc.vector.memset(warm[:], 1.0)
            nc.scalar.activation(warm[:, 0:1], warm[:, 0:1], AF.Exp)
            nc.scalar.activation(warm[:, 1:2], warm[:, 1:2], AF.Ln)

            # ---- input loads (x first, segmented: GroupNorm stats overlap) ----
            tx = data.tile([C, N], F32R)
            for i in range(NSEG):
                nc.sync.dma_start(
                    tx[:, SEG * i : SEG * (i + 1)], x_d[:, SEG * i : SEG * (i + 1)]
                )
            txf = tx[:].bitcast(F32)

            wqT = const.tile([C, C], F32R)
            wkT = const.tile([C, C], F32R)
            wvT = const.tile([C, C], F32R)
            woT = const.tile([C, C], F32R)
            gmat = const.tile([C, GROUPS], F32R)
            gexp = const.tile([GROUPS, C], F32R)
            ident = const.tile([C, C], BF16)
            gnw = const.tile([C, 1], F32)
            gnb = const.tile([C, 1], F32)
            bq = const.tile([C, 1], F32)
            bk = const.tile([C, 1], F32)
            bv = const.tile([C, 1], F32)
            bo = const.tile([C, 1], F32)
            for t, d in [
                (gmat, gmat_d), (gexp, gexp_d), (gnw, gnw_d), (gnb, gnb_d),
                (wqT, wqT_d), (wkT, wkT_d), (wvT, wvT_d), (woT, woT_d),
                (ident, ident_d),
                (bq, bq_d), (bk, bk_d), (bv, bv_d), (bo, bo_d),
            ]:
                nc.sync.dma_start(t[:], d[:])

            # ---- GroupNorm statistics: bn_stats per DMA segment ----
            bstats = small.tile([C, NSEG, 6], F32)
            for i in range(NSEG):
                nc.vector.bn_stats(
                    bstats[:, i, :], txf[:, SEG * i : SEG * (i + 1)]
                )
            mv = small.tile([C, 2], F32)  # per-channel (mean, var)
            nc.vector.bn_aggr(mv[:], bstats[:].rearrange("p g f -> p (g f)"))

            # per-channel (mean, E[x^2]); group-combine via gmat matmul
            mex = small.tile([C, 2], F32)
            nc.vector.tensor_mul(mex[:, 1:2], mv[:, 0:1], mv[:, 0:1])
            nc.vector.tensor_add(mex[:, 1:2], mex[:, 1:2], mv[:, 1:2])
            nc.vector.tensor_copy(mex[:, 0:1], mv[:, 0:1])
            mex_r = small.tile([C, 2], F32R)
            nc.vector.tensor_copy(mex_r[:], mex[:])

            with tc.tile_pool(name="gnps", bufs=1, space="PSUM") as gnps:
                psg = gnps.tile([GROUPS, 2], F32)
                nc.tensor.matmul(psg[:], gmat[:], mex_r[:], start=True, stop=True)

                inv_g = 1.0 / GSIZE
                t32 = small.tile([GROUPS, 4], F32)
                nc.vector.tensor_scalar_mul(t32[:, 0:1], psg[:, 0:1], inv_g)
                nc.vector.tensor_scalar_mul(t32[:, 1:2], psg[:, 1:2], inv_g)
                nc.vector.tensor_mul(t32[:, 2:3], t32[:, 0:1], t32[:, 0:1])
                nc.vector.tensor_sub(t32[:, 3:4], t32[:, 1:2], t32[:, 2:3])
                eps_t = small.tile([GROUPS, 1], F32)
                nc.vector.memset(eps_t[:], EPS)
                nc.scalar.activation(t32[:, 2:3], t32[:, 3:4], AF.Ln, bias=eps_t[:])
                rstd_f = small.tile([GROUPS, 1], F32)
                nc.scalar.activation(rstd_f[:], t32[:, 2:3], AF.Exp, scale=-0.5)
                mstat = small.tile([GROUPS, 2], F32R)
                nc.vector.tensor_copy(mstat[:, 0:1], t32[:, 0:1])
                nc.vector.tensor_copy(mstat[:, 1:2], rstd_f[:])

                pse = gnps.tile([C, 2], F32)
                nc.tensor.matmul(pse[:], gexp[:], mstat[:], start=True, stop=True)

                A_sb = small.tile([C, 1], F32)
                B_sb = small.tile([C, 1], F32)
                nc.vector.tensor_mul(A_sb[:], pse[:, 1:2], gnw[:])
                nc.vector.tensor_mul(B_sb[:], pse[:, 0:1], A_sb[:])
                nc.vector.tensor_sub(B_sb[:], gnb[:], B_sb[:])

            # ---- fold the GroupNorm affine into the projection weights:
            # ---- q = Wq(A*x + B) + bq = (Wq diag(A)) x + (Wq B + bq)
            wq2 = small.tile([C, C], F32R)
            wk2 = small.tile([C, C], F32R)
            wv2 = small.tile([C, C], F32R)
            bq2 = small.tile([C, 1], F32)
            bk2 = small.tile([C, 1], F32)
            bv2 = small.tile([C, 1], F32)
            with tc.tile_pool(name="foldps", bufs=1, space="PSUM") as foldps:
                psb = foldps.tile([C, 4], F32)
                for i, (wT, w2, bias, b2) in enumerate((
                    (wqT, wq2, bq, bq2),
                    (wkT, wk2, bk, bk2),
                    (wvT, wv2, bv, bv2),
                )):
                    nc.vector.tensor_scalar_mul(w2[:], wT[:], A_sb[:])
                    nc.tensor.matmul(
                        psb[:, i : i + 1], wT[:].bitcast(F32), B_sb[:],
                        start=True, stop=True,
                    )
                    nc.vector.tensor_add(b2[:], psb[:, i : i + 1], bias[:])

            # ---- Q/K/V projections; q,k first so the attention loop can
            # ---- start while v/vT still evacuates ----
            q = data.tile([C, N], BF16)
            k = data.tile([C, N], BF16)
            v = data.tile([C, N], BF16)
            with tc.tile_pool(name="projps", bufs=2, space="PSUM") as projps:
                for g in range(NQG):
                    for wT, bias, dst, eng in (
                        (wq2, bq2, q, "scalar"),
                        (wk2, bk2, k, "scalar"),
                        (wv2, bv2, v, "vector"),
                    ):
                        pp = projps.tile([C, QG], F32, tag="pp")
                        o = g * QG
                        nc.tensor.matmul(
                            pp[:, 0:512], wT[:], tx[:, o : o + 512],
                            start=True, stop=True,
                        )
                        nc.tensor.matmul(
                            pp[:, 512:QG], wT[:], tx[:, o + 512 : o + QG],
                            start=True, stop=True,
                        )
                        outv = dst[:, o : o + QG]
                        if eng == "scalar":
                            nc.scalar.activation(outv, pp[:], AF.Identity, bias=bias[:])
                        else:
                            nc.vector.tensor_scalar(
                                outv, pp[:], bias[:], None, op0=OP.add
                            )

                vT = data.tile([C, NCHUNK * C], BF16)
                for grp in range(0, NCHUNK, 4):
                    cnt = min(4, NCHUNK - grp)
                    pvt = projps.tile([C, 512], BF16, tag="pvt")
                    for j in range(cnt):
                        ch = grp + j
                        nc.tensor.transpose(
                            pvt[:, 128 * j : 128 * (j + 1)],
                            v[:, 128 * ch : 128 * (ch + 1)],
                            ident[:],
                        )
                    nc.vector.tensor_copy(
                        vT[:, 128 * grp : 128 * (grp + cnt)], pvt[:, : 128 * cnt]
                    )

            # ---- main attention loop ----
            pair_tiles = [None] * NPAIR
            e_tiles = [None] * NCHUNK
            ctx_all = data.tile([C, N], F32R)
            with (
                tc.tile_pool(name="ctxps", bufs=1, space="PSUM") as ctxps,
                tc.tile_pool(name="sps", bufs=2, space="PSUM") as sps,
            ):
                ctx_ps = [
                    ctxps.tile([C, 512], F32, tag=f"ctx{i}", name=f"ctx_ps{i}")
                    for i in range(len(CTX_LIVE))
                ]

                def emit_av(ch, part):
                    ec = e_tiles[ch]
                    for i in ([0, 1], [2], [3])[part]:
                        o = CTX_LIVE[i]
                        nc.tensor.matmul(
                            ctx_ps[i][:, :],
                            vT[:, 128 * ch : 128 * (ch + 1)],
                            ec[:, o : o + 512],
                            start=(ch == 0),
                            stop=(ch == NCHUNK - 1),
                        )

                for it in range(NCHUNK + 4):
                    ch = it if it < NCHUNK else None
                    av = it - 4
                    if ch is not None:
                        pj = ch // 2
                        if ch % 2 == 0:
                            pair_tiles[pj] = epool.tile(
                                [C, 2 * N], BF16, tag="E", name=f"E_{pj}"
                            )
                        ec = pair_tiles[pj][:, (ch % 2) * N : (ch % 2) * N + N]
                        e_tiles[ch] = ec
                        klhs = k[:, 128 * ch : 128 * (ch + 1)]
                        for g in range(NQG):
                            ps = sps.tile([C, QG], F32, tag="spsum")
                            o = g * QG
                            nc.tensor.matmul(
                                ps[:, 0:512], klhs, q[:, o : o + 512],
                                start=True, stop=True,
                            )
                            nc.tensor.matmul(
                                ps[:, 512:QG], klhs, q[:, o + 512 : o + QG],
                                start=True, stop=True,
                            )
                            nc.scalar.activation(ec[:, o : o + QG], ps[:, :], AF.Exp)
                            if av >= 0:
                                emit_av(av, g)
                    else:
                        for g in range(NQG):
                            emit_av(av, g)

                    if ch is None or ch % 2 == 0:
                        continue
                    # ---- softmax denominators + normalize, once per pair ----
                    pj = ch // 2
                    ep = pair_tiles[pj]
                    # grouped reduce over h (contiguous inner, w-major layout)
                    e4 = ep[:].rearrange("p (c w h) -> p c w h", c=2, h=H)
                    dsum = soft.tile([C, 2 * W], F32, tag="D")
                    nc.vector.tensor_reduce(
                        dsum[:].rearrange("p (c w) -> p c w", c=2),
                        e4, axis=mybir.AxisListType.X, op=OP.add,
                    )
                    rden = soft.tile([C, 2 * W], F32, tag="R")
                    nc.vector.reciprocal_approx_fast(rden[:], dsum[:])
                    # duplicate into adjacent bf16 pairs on ScalarE: rp[cw, t]
                    rpair = soft.tile([C, 2 * W, 2], BF16, tag="RP")
                    nc.scalar.copy(
                        rpair[:], rden[:, :, None].to_broadcast([C, 2 * W, 2])
                    )
                    # normalize per chunk: [p, w, h2, t] *= rpair[p, w-slice, t]
                    # (innermost step-1 pair keeps the DVE in 2x mode)
                    for cc in (ch - 1, ch):
                        evv = e_tiles[cc].rearrange(
                            "p (w h2 t) -> p w h2 t", h2=H // 2, t=2
                        )
                        side = cc % 2
                        rb = rpair[:, side * W : (side + 1) * W, :][
                            :, :, None, :
                        ].to_broadcast([C, W, H // 2, 2])
                        if pj in MUL_VE_PAIRS:
                            nc.vector.tensor_tensor(
                                out=evv, in0=evv, in1=rb, op=OP.mult
                            )
                        else:
                            nc.gpsimd.tensor_tensor(
                                out=evv, in0=evv, in1=rb, op=OP.mult
                            )

                for i, o in enumerate(CTX_LIVE):
                    nc.scalar.copy(ctx_all[:, o : o + 512], ctx_ps[i][:, :])

            # ---- ctx tail (columns 2048:2304) + output projection + residual ----
            out_nat = data.tile([C, N], F32)
            with tc.tile_pool(name="ops", bufs=2, space="PSUM") as ops:
                tail = ops.tile([C, TAIL_SZ], F32, tag="tail")
                for ch in range(NCHUNK):
                    nc.tensor.matmul(
                        tail[:, :],
                        vT[:, 128 * ch : 128 * (ch + 1)],
                        e_tiles[ch][:, TAIL_OFF : TAIL_OFF + TAIL_SZ],
                        start=(ch == 0),
                        stop=(ch == NCHUNK - 1),
                    )
                nc.scalar.copy(ctx_all[:, TAIL_OFF : TAIL_OFF + TAIL_SZ], tail[:, :])

                for g in range(NQG):
                    po = ops.tile([C, QG], F32, tag="po", name=f"po_{g}")
                    o = g * QG
                    nc.tensor.matmul(
                        po[:, 0:512], woT[:], ctx_all[:, o : o + 512],
                        start=True, stop=True,
                    )
                    nc.tensor.matmul(
                        po[:, 512:QG], woT[:], ctx_all[:, o + 512 : o + QG],
                        start=True, stop=True,
                    )
                    # fused bias + residual: out = (po + bo) + x
                    nc.vector.scalar_tensor_tensor(
                        out_nat[:, o : o + QG], po[:], bo[:],
                        txf[:, o : o + QG], op0=OP.add, op1=OP.add,
                    )
                    nc.sync.dma_start(
                        out_d[:, o : o + QG], out_nat[:, o : o + QG]
                    )

    nc.compile()
    return nc


_PROGRAM_CACHE = None


def kernel(**inputs: np.ndarray) -> np.ndarray:
    global _PROGRAM_CACHE
    if _PROGRAM_CACHE is None:
        _PROGRAM_CACHE = _build_program()
    nc = _PROGRAM_CACHE

    import ml_dtypes

    f32 = lambda a: np.ascontiguousarray(np.asarray(a), dtype=np.float32)
    x = f32(inputs["x"])
    scale = 1.0 / np.sqrt(np.float32(C))

    gmat = np.zeros((C, GROUPS), np.float32)
    gmat[np.arange(C), np.arange(C) // GSIZE] = 1.0

    shared = {
        "gn_w": f32(inputs["gn_w"]).reshape(C, 1),
        "gn_b": f32(inputs["gn_b"]).reshape(C, 1),
        "wqT": np.ascontiguousarray(f32(inputs["wq"]).T * scale),
        "wkT": np.ascontiguousarray(f32(inputs["wk"]).T),
        "wvT": np.ascontiguousarray(f32(inputs["wv"]).T),
        "woT": np.ascontiguousarray(f32(inputs["wo"]).T),
        "bq": f32(inputs["bq"]).reshape(C, 1) * scale,
        "bk": f32(inputs["bk"]).reshape(C, 1),
        "bv": f32(inputs["bv"]).reshape(C, 1),
        "bo": f32(inputs["bo"]).reshape(C, 1),
        "gmat": gmat,
        "gexp": np.ascontiguousarray(gmat.T),
        "ident": np.eye(C).astype(ml_dtypes.bfloat16),
    }
    # host-side w-major relabeling: device sees x with q' = w*48 + h
    in_maps = [
        {
            **shared,
            "x": np.ascontiguousarray(
                x[b].reshape(C, H, W).transpose(0, 2, 1).reshape(C, N)
            ),
        }
        for b in range(B)
    ]

    res = bass_utils.run_bass_kernel_spmd(nc, in_maps, core_ids=list(range(NCORES)))
    # un-permute: device output is [C, (w h)] -> [C, H, W]
    out = np.stack(
        [res.results[b]["out"].reshape(C, W, H).transpose(0, 2, 1) for b in range(B)]
    )
    return np.ascontiguousarray(out).astype(np.float32)
